# revision 11
# baseline (speedup 1.0000x reference)
"""2-layer GAT forward + CE loss on 8 Trainium2 NeuronCores (Bass/Tile).

Sharding: destinations are bin-packed into 168 bins (21 per core, <=128 dsts
and <=T*128 edges per bin). Each core computes its own nodes' features
(x @ W1ext), tables are AllGathered, then each core runs the edge phase
(gather-softmax-aggregate via dma_gather + indicator matmuls) for its bins.
"""

import heapq
import os

import numpy as np

import concourse.bacc as bacc
import concourse.bass as bass
import concourse.mybir as mybir
import concourse.tile as tile
from concourse.bass_utils import run_bass_kernel_spmd

P = 128
NCORES = 8
N = 20000
E_RAW = 100000
D = 512
H1, C1 = 8, 64
H2, C2 = 4, 128
BINS_PER_CORE = 21
NBINS = NCORES * BINS_PER_CORE          # 168
SLOTS_PER_CORE = BINS_PER_CORE * P      # 2688
NSLOTS = NCORES * SLOTS_PER_CORE        # 21504
GRP = 1   # bins per score-gather group (dma_gather breaks above ~768 idxs at elem 64)
NGRP = BINS_PER_CORE // GRP             # 7

dt = mybir.dt

LAST_EXEC_NS = [None]


# ---------------------------------------------------------------- host prep

def _swizzle_idx(idx, J):
    """Pack J*128 indices into the dma_gather int16 SBUF layout [128, 8J].

    HW mapping: out[16s+q, j] = table[idx16[q, 8j+s]]  (s in 0..7, q in 0..15)
    """
    assert idx.shape[0] == J * P
    sw = idx.reshape(J, 8, 16).transpose(2, 0, 1).reshape(16, 8 * J)
    return np.tile(sw.astype(np.int16), (8, 1))


def _prep(street_embedding, edge_index, y, train_mask, W1, att_src1, att_dst1,
          b1, W2, att_src2, att_dst2, b2):
    x = np.asarray(street_embedding, np.float32)
    ei = np.asarray(edge_index, np.int64)
    y = np.asarray(y, np.int64)
    mask = np.asarray(train_mask)

    src = np.concatenate([ei[0], np.arange(N, dtype=np.int64)])
    dst = np.concatenate([ei[1], np.arange(N, dtype=np.int64)])
    deg = np.bincount(dst, minlength=N)

    # --- bin packing: nodes (as dsts) -> bins, balancing edge counts ---
    order = np.argsort(-deg, kind="stable")
    heap = [(0, b) for b in range(NBINS)]
    heapq.heapify(heap)
    bin_of = np.empty(N, np.int32)
    bin_nodes = [[] for _ in range(NBINS)]
    bin_load = np.zeros(NBINS, np.int64)
    for v in order:
        while True:
            load, b = heapq.heappop(heap)
            if len(bin_nodes[b]) < P:
                break
        bin_of[v] = b
        bin_nodes[b].append(v)
        bin_load[b] = load + deg[v]
        heapq.heappush(heap, (int(bin_load[b]), b))

    T = int(np.ceil(bin_load.max() / P))
    pos_of = np.empty(N, np.int32)
    node_of_slot = np.full(NSLOTS, -1, np.int32)
    for b in range(NBINS):
        for i, v in enumerate(bin_nodes[b]):
            pos_of[v] = i
            core, lb = b // BINS_PER_CORE, b % BINS_PER_CORE
            node_of_slot[core * SLOTS_PER_CORE + lb * P + i] = v
    slot_of = np.empty(N, np.int32)
    valid_slots = node_of_slot >= 0
    slot_of[node_of_slot[valid_slots]] = np.nonzero(valid_slots)[0]

    # --- edges grouped by bin, padded to T*128 each ---
    eb = bin_of[dst]
    eorder = np.argsort(eb, kind="stable")
    s_srt, d_srt, eb_srt = src[eorder], dst[eorder], eb[eorder]
    starts = np.searchsorted(eb_srt, np.arange(NBINS))
    ends = np.searchsorted(eb_srt, np.arange(NBINS) + 1)

    EP = T * P
    src_slot = np.zeros((NBINS, EP), np.int32)
    dst_slot = np.zeros((NBINS, EP), np.int32)
    dstloc = np.zeros((NBINS, EP), np.float32)
    validm = np.zeros((NBINS, EP), np.float32)
    for b in range(NBINS):
        s, e = starts[b], ends[b]
        n = e - s
        src_slot[b, :n] = slot_of[s_srt[s:e]]
        dst_slot[b, :n] = slot_of[d_srt[s:e]]
        dstloc[b, :n] = pos_of[d_srt[s:e]]
        validm[b, :n] = 1.0

    per_core = []
    for c in range(NCORES):
        b0 = c * BINS_PER_CORE
        idx_feat = np.stack([_swizzle_idx(src_slot[b0 + b], T)
                             for b in range(BINS_PER_CORE)])
        idx_feat = idx_feat.transpose(1, 0, 2).reshape(P, BINS_PER_CORE * 8 * T)
        gs, gd = [], []
        for g in range(NGRP):
            ss = src_slot[b0 + g * GRP: b0 + (g + 1) * GRP].reshape(-1)
            dd = dst_slot[b0 + g * GRP: b0 + (g + 1) * GRP].reshape(-1)
            gs.append(_swizzle_idx(ss, GRP * T))
            gd.append(_swizzle_idx(dd, GRP * T))
        idx_ssrc = np.stack(gs).transpose(1, 0, 2).reshape(P, NGRP * 8 * GRP * T)
        idx_sdst = np.stack(gd).transpose(1, 0, 2).reshape(P, NGRP * 8 * GRP * T)

        dl = dstloc[b0:b0 + BINS_PER_CORE].reshape(BINS_PER_CORE, T, P)
        dl = dl.transpose(2, 0, 1).reshape(P, BINS_PER_CORE * T)
        vm = validm[b0:b0 + BINS_PER_CORE].reshape(BINS_PER_CORE, T, P)
        vm = vm.transpose(2, 0, 1).reshape(P, BINS_PER_CORE * T)

        nos = node_of_slot[c * SLOTS_PER_CORE:(c + 1) * SLOTS_PER_CORE]
        ok = nos >= 0
        xs = np.zeros((SLOTS_PER_CORE, D), np.float32)
        xs[ok] = x[nos[ok]]
        xT4 = xs.reshape(SLOTS_PER_CORE, 4, P).transpose(2, 1, 0).copy()
        wm = np.zeros(SLOTS_PER_CORE, np.float32)
        wm[ok] = mask[nos[ok]].astype(np.float32)
        yl = np.zeros(SLOTS_PER_CORE, np.float32)
        yl[ok] = y[nos[ok]].astype(np.float32)
        wm = wm.reshape(BINS_PER_CORE, P).T.copy()
        yl = yl.reshape(BINS_PER_CORE, P).T.copy()
        per_core.append(dict(xT4=xT4, idxF=idx_feat, idxS=idx_ssrc,
                             idxD=idx_sdst, dstloc=dl, valid=vm,
                             wmask=wm, yloc=yl))

    W1 = np.asarray(W1, np.float32)
    W2 = np.asarray(W2, np.float32)
    A1s = np.einsum("khc,hc->kh", W1.reshape(D, H1, C1),
                    np.asarray(att_src1, np.float32))
    A1d = np.einsum("khc,hc->kh", W1.reshape(D, H1, C1),
                    np.asarray(att_dst1, np.float32))
    A2s = np.einsum("khc,hc->kh", W2.reshape(D, H2, C2),
                    np.asarray(att_src2, np.float32))
    A2d = np.einsum("khc,hc->kh", W2.reshape(D, H2, C2),
                    np.asarray(att_dst2, np.float32))
    W1e = np.concatenate([W1, A1s, A1d], 1).reshape(4, P, D + 2 * H1)
    W1e = W1e.transpose(1, 0, 2).copy()  # [P, 4, 528] partition-first
    W2e = np.concatenate([W2, A2s, A2d], 1).reshape(4, P, D + 2 * H2)
    W2e = W2e.transpose(1, 0, 2).copy()  # [P, 4, 520]

    shared = dict(
        W1e=W1e.copy(), W2e=W2e.copy(),
        b1row=np.asarray(b1, np.float32).reshape(1, D),
        b2row=np.asarray(b2, np.float32).reshape(1, C2),
        iota=np.tile(np.arange(P, dtype=np.float32), (P, 1)),
        ident=np.eye(P, dtype=np.float32),
    )
    meta = dict(T=T, node_of_slot=node_of_slot,
                mask_sum=np.float32(mask.astype(np.float64).sum()))
    return per_core, shared, meta


# ---------------------------------------------------------------- device code

def _edge_layer(nc, sb, psb, psd, layer, T, feat_full, score_full, consts,
                out_cb):
    """Edge phase for one GAT layer.

    layer 1: H=8, C=64, score cols asrc=0:8 adst=8:16
    layer 2: H=4, C=128, score cols asrc=0:4 adst=4:8
    out_cb(nb, ps_out, rec) consumes the per-bin PSUM accumulator + recip.
    """
    H = H1 if layer == 1 else H2
    C = C1 if layer == 1 else C2
    sub = os.environ.get("GAT_SUB", "full")
    iota, idxS, idxD, idxF, dstloc, valid = consts
    sA = 0   # asrc columns start
    dA = H   # adst columns start

    ex_tiles = []
    for g in range(NGRP):
        J = GRP * T
        gs = sb.tile([P, J, 64], dt.float32, tag="gs")
        gd = sb.tile([P, J, 64], dt.float32, tag="gd")
        if sub == "nog":
            nc.gpsimd.memset(gs[:], 0.1)
            nc.gpsimd.memset(gd[:], 0.1)
        else:
            nc.gpsimd.dma_gather(gs[:], score_full[:],
                                 idxS[:, g * 8 * J:(g + 1) * 8 * J],
                                 J * P, J * P, 64)
            nc.gpsimd.dma_gather(gd[:], score_full[:],
                                 idxD[:, g * 8 * J:(g + 1) * 8 * J],
                                 J * P, J * P, 64)
        ssum = sb.tile([P, J, H], dt.float32, tag="ssum")
        nc.vector.tensor_tensor(out=ssum[:], in0=gs[:, :, sA:sA + H],
                                in1=gd[:, :, dA:dA + H],
                                op=mybir.AluOpType.add)
        slr = sb.tile([P, J, H], dt.float32, tag="slr")
        nc.vector.tensor_scalar(out=slr[:], in0=ssum[:], scalar1=0.2,
                                scalar2=None, op0=mybir.AluOpType.mult)
        nc.vector.tensor_tensor(out=slr[:], in0=slr[:], in1=ssum[:],
                                op=mybir.AluOpType.max)
        exg = sb.tile([P, J, H], dt.float32, tag=f"ex{g}")
        nc.scalar.activation(exg[:], slr[:], mybir.ActivationFunctionType.Exp)
        vslice = valid[:, (g * GRP) * T:(g + 1) * GRP * T]
        nc.vector.tensor_tensor(out=exg[:], in0=exg[:],
                                in1=vslice[:, :, None].to_broadcast([P, J, H]),
                                op=mybir.AluOpType.mult)
        ex_tiles.append(exg)
    if sub == "sg":
        return

    for nb in range(BINS_PER_CORE):
        g, lb = nb // GRP, nb % GRP
        exg = ex_tiles[g]
        fg = sb.tile([P, T, D], dt.float32, tag="fg")
        nc.gpsimd.dma_gather(fg[:], feat_full[:],
                             idxF[:, nb * 8 * T:(nb + 1) * 8 * T],
                             T * P, T * P, D)
        if sub == "fg":
            continue
        ps_out = psb.tile([P, D], dt.float32, space="PSUM", tag="big")
        ps_den = psd.tile([P, H], dt.float32, space="PSUM", tag="ps_den")
        for t in range(T):
            ind = sb.tile([P, P], dt.float32, tag="ind")
            nc.vector.tensor_scalar(out=ind[:], in0=iota[:],
                                    scalar1=dstloc[:, nb * T + t:nb * T + t + 1],
                                    scalar2=None, op0=mybir.AluOpType.is_equal)
            ext = exg[:, lb * T + t, :]
            wf = sb.tile([P, H, C], dt.float32, tag="wf")
            hh = H // 2
            nc.vector.tensor_tensor(
                out=wf[:, :hh, :],
                in0=fg[:, t].rearrange("p (h c) -> p h c", h=H)[:, :hh, :],
                in1=ext[:, :hh, None].to_broadcast([P, hh, C]),
                op=mybir.AluOpType.mult)
            for h in range(hh, H):
                nc.scalar.activation(wf[:, h, :], fg[:, t, h * C:(h + 1) * C],
                                     mybir.ActivationFunctionType.Copy,
                                     scale=ext[:, h:h + 1])
            if sub == "nomm":
                continue
            nc.tensor.matmul(ps_den[:], lhsT=ind[:], rhs=ext[:],
                             start=(t == 0), stop=(t == T - 1))
            nc.tensor.matmul(ps_out[:], lhsT=ind[:],
                             rhs=wf[:].rearrange("p h c -> p (h c)"),
                             start=(t == 0), stop=(t == T - 1))
        if sub == "nomm":
            continue
        den = sb.tile([P, H], dt.float32, tag="den")
        nc.vector.tensor_scalar(out=den[:], in0=ps_den[:], scalar1=1e-16,
                                scalar2=None, op0=mybir.AluOpType.add)
        rec = sb.tile([P, H], dt.float32, tag="rec")
        nc.vector.reciprocal(rec[:], den[:])
        out_cb(nb, ps_out, rec)


def _build(T):
    stage = int(os.environ.get("GAT_STAGE", "4"))
    nc = bacc.Bacc("TRN2", target_bir_lowering=False, debug=False,
                   num_devices=NCORES)

    def inp(name, shape, d=dt.float32):
        return nc.dram_tensor(name, shape, d, kind="ExternalInput").ap()

    xT4 = inp("xT4", [P, 4, SLOTS_PER_CORE])
    W1e = inp("W1e", [P, 4, D + 2 * H1])
    W2e = inp("W2e", [P, 4, D + 2 * H2])
    b1row = inp("b1row", [1, D])
    b2row = inp("b2row", [1, C2])
    iota_in = inp("iota", [P, P])
    ident_in = inp("ident", [P, P])
    idxF_in = inp("idxF", [P, BINS_PER_CORE * 8 * T], dt.int16)
    idxS_in = inp("idxS", [P, NGRP * 8 * GRP * T], dt.int16)
    idxD_in = inp("idxD", [P, NGRP * 8 * GRP * T], dt.int16)
    dstloc_in = inp("dstloc", [P, BINS_PER_CORE * T])
    valid_in = inp("valid", [P, BINS_PER_CORE * T])
    wmask_in = inp("wmask", [P, BINS_PER_CORE])
    yloc_in = inp("yloc", [P, BINS_PER_CORE])

    out2_dram = nc.dram_tensor("out2", [SLOTS_PER_CORE, C2], dt.float32,
                               kind="ExternalOutput").ap()
    wce_dram = nc.dram_tensor("wce", [P, BINS_PER_CORE], dt.float32,
                              kind="ExternalOutput").ap()

    with tile.TileContext(nc) as tc:
        with (
            tc.tile_pool(name="cst", bufs=1) as cst,
            tc.tile_pool(name="sb", bufs=2) as sb,
            tc.tile_pool(name="psb", bufs=3, space="PSUM") as psb,
            tc.tile_pool(name="psd", bufs=2, space="PSUM") as psd,
            tc.tile_pool(name="ps1", bufs=1, space="PSUM") as ps1,
            tc.tile_pool(name="dram", bufs=1, space="DRAM") as dp,
        ):
            # ---- constants to SBUF ----
            def load(name, ap_in, shape, d=dt.float32):
                t = cst.tile(shape, d, tag=name)
                nc.sync.dma_start(t[:], ap_in[:])
                return t

            W1s = load("W1s", W1e, [P, 4, D + 2 * H1])
            W2s = load("W2s", W2e, [P, 4, D + 2 * H2])
            iota = load("iota", iota_in, [P, P])
            ident = load("ident", ident_in, [P, P])
            idxF = load("idxF", idxF_in, [P, BINS_PER_CORE * 8 * T], dt.int16)
            idxS = load("idxS", idxS_in, [P, NGRP * 8 * GRP * T], dt.int16)
            idxD = load("idxD", idxD_in, [P, NGRP * 8 * GRP * T], dt.int16)
            dstloc = load("dstloc", dstloc_in, [P, BINS_PER_CORE * T])
            valid = load("valid", valid_in, [P, BINS_PER_CORE * T])
            wmask = load("wmask", wmask_in, [P, BINS_PER_CORE])
            yloc = load("yloc", yloc_in, [P, BINS_PER_CORE])
            b1t = cst.tile([P, D], dt.float32)
            nc.sync.dma_start(b1t[:], b1row[:].partition_broadcast(P))
            b2t = cst.tile([P, C2], dt.float32)
            nc.sync.dma_start(b2t[:], b2row[:].partition_broadcast(P))

            # ---- DRAM tables ----
            feat1_sh = dp.tile([SLOTS_PER_CORE, D], dt.float32)
            sc1_sh = dp.tile([SLOTS_PER_CORE, 64], dt.float32)
            feat2_sh = dp.tile([SLOTS_PER_CORE, D], dt.float32)
            sc2_sh = dp.tile([SLOTS_PER_CORE, 64], dt.float32)
            feat1_f = dp.tile([NSLOTS, D], dt.float32, addr_space="Shared")
            sc1_f = dp.tile([NSLOTS, 64], dt.float32, addr_space="Shared")
            feat2_f = dp.tile([NSLOTS, D], dt.float32, addr_space="Shared")
            sc2_f = dp.tile([NSLOTS, 64], dt.float32, addr_space="Shared")

            # ---- stage A: own-shard xp1 = x @ W1ext ----
            for nb in range(BINS_PER_CORE):
                xt = sb.tile([P, 4, P], dt.float32, tag="xt")
                nc.sync.dma_start(xt[:], xT4[:, :, nb * P:(nb + 1) * P])
                psA = psb.tile([P, D], dt.float32, space="PSUM", tag="big")
                psB = ps1.tile([P, 2 * H1], dt.float32, space="PSUM", tag="psB")
                for k in range(4):
                    nc.tensor.matmul(psA[:], lhsT=xt[:, k, :],
                                     rhs=W1s[:, k, :D],
                                     start=(k == 0), stop=(k == 3))
                    nc.tensor.matmul(psB[:], lhsT=xt[:, k, :],
                                     rhs=W1s[:, k, D:],
                                     start=(k == 0), stop=(k == 3))
                fo = sb.tile([P, D], dt.float32, tag="fo")
                nc.vector.tensor_copy(fo[:], psA[:])
                so = sb.tile([P, 2 * H1], dt.float32, tag="so")
                nc.scalar.activation(so[:], psB[:],
                                     mybir.ActivationFunctionType.Copy)
                nc.sync.dma_start(feat1_sh[nb * P:(nb + 1) * P, :], fo[:])
                nc.sync.dma_start(sc1_sh[nb * P:(nb + 1) * P, :2 * H1], so[:])

            # ---- AllGather layer-1 tables ----
            rg = [list(range(NCORES))]
            nc.gpsimd.collective_compute(
                "AllGather", mybir.AluOpType.bypass, replica_groups=rg,
                ins=[sc1_sh.opt()], outs=[sc1_f.opt()])
            nc.gpsimd.collective_compute(
                "AllGather", mybir.AluOpType.bypass, replica_groups=rg,
                ins=[feat1_sh.opt()], outs=[feat1_f.opt()])
            if os.environ.get("GAT_BAR", "0") == "1":
                tc.strict_bb_all_engine_barrier()

            if stage == 1:
                tmp = sb.tile([P, C2], dt.float32, tag="tmp1")
                nc.sync.dma_start(tmp[:], feat1_f[:P, :C2])
                nc.sync.dma_start(out2_dram[:P, :], tmp[:])
                tmp2 = sb.tile([P, BINS_PER_CORE], dt.float32, tag="tmp2")
                nc.sync.dma_start(tmp2[:], sc1_f[:P, :BINS_PER_CORE])
                nc.sync.dma_start(wce_dram[:], tmp2[:])

            # ---- layer-1 edge phase + inline stage D ----
            consts = (iota, idxS, idxD, idxF, dstloc, valid)

            def l1_out(nb, ps_out, rec):
                hblk = sb.tile([P, D], dt.float32, tag="hblk")
                nc.vector.tensor_tensor(
                    out=hblk[:].rearrange("p (h c) -> p h c", h=H1),
                    in0=ps_out[:].rearrange("p (h c) -> p h c", h=H1),
                    in1=rec[:, :, None].to_broadcast([P, H1, C1]),
                    op=mybir.AluOpType.mult)
                nc.vector.tensor_tensor(out=hblk[:], in0=hblk[:], in1=b1t[:],
                                        op=mybir.AluOpType.add)
                # ELU
                e1 = sb.tile([P, D], dt.float32, tag="e1")
                nc.vector.tensor_scalar(out=e1[:], in0=hblk[:], scalar1=0.0,
                                        scalar2=None, op0=mybir.AluOpType.min)
                nc.scalar.activation(e1[:], e1[:],
                                     mybir.ActivationFunctionType.Exp)
                nc.vector.tensor_scalar(out=hblk[:], in0=hblk[:], scalar1=0.0,
                                        scalar2=None, op0=mybir.AluOpType.max)
                nc.vector.tensor_tensor(out=hblk[:], in0=hblk[:], in1=e1[:],
                                        op=mybir.AluOpType.add)
                nc.vector.tensor_scalar(out=hblk[:], in0=hblk[:], scalar1=1.0,
                                        scalar2=None,
                                        op0=mybir.AluOpType.subtract)
                if stage == 2:
                    nc.sync.dma_start(out2_dram[nb * P:(nb + 1) * P, :],
                                      hblk[:, :C2])
                    return
                # stage D: xp2 rows for own nodes
                ps2A = psb.tile([P, D], dt.float32, space="PSUM", tag="big")
                ps2B = ps1.tile([P, 2 * H2], dt.float32, space="PSUM",
                                tag="ps2B")
                for k in range(4):
                    pst = ps1.tile([P, P], dt.float32, space="PSUM", tag="pst")
                    nc.tensor.transpose(pst[:], hblk[:, k * P:(k + 1) * P],
                                        ident[:])
                    hT = sb.tile([P, P], dt.float32, tag="hT")
                    nc.scalar.activation(hT[:], pst[:],
                                         mybir.ActivationFunctionType.Copy)
                    nc.tensor.matmul(ps2A[:], lhsT=hT[:], rhs=W2s[:, k, :D],
                                     start=(k == 0), stop=(k == 3))
                    nc.tensor.matmul(ps2B[:], lhsT=hT[:], rhs=W2s[:, k, D:],
                                     start=(k == 0), stop=(k == 3))
                f2 = sb.tile([P, D], dt.float32, tag="f2")
                nc.vector.tensor_copy(f2[:], ps2A[:])
                s2 = sb.tile([P, 2 * H2], dt.float32, tag="s2")
                nc.scalar.activation(s2[:], ps2B[:],
                                     mybir.ActivationFunctionType.Copy)
                nc.sync.dma_start(feat2_sh[nb * P:(nb + 1) * P, :], f2[:])
                nc.sync.dma_start(sc2_sh[nb * P:(nb + 1) * P, :2 * H2], s2[:])

            if stage >= 2:
                _edge_layer(nc, sb, psb, psd, 1, T, feat1_f, sc1_f, consts,
                            l1_out)
            if stage == 25:
                tmp = sb.tile([P, C2], dt.float32, tag="tmp1")
                nc.sync.dma_start(tmp[:], feat2_sh[:P, :C2])
                nc.sync.dma_start(out2_dram[:P, :], tmp[:])

            # ---- AllGather layer-2 tables ----
            if stage >= 3:
                nc.gpsimd.collective_compute(
                    "AllGather", mybir.AluOpType.bypass, replica_groups=rg,
                    ins=[sc2_sh.opt()], outs=[sc2_f.opt()])
                nc.gpsimd.collective_compute(
                    "AllGather", mybir.AluOpType.bypass, replica_groups=rg,
                    ins=[feat2_sh.opt()], outs=[feat2_f.opt()])
                if os.environ.get("GAT_BAR", "0") == "1":
                    tc.strict_bb_all_engine_barrier()

            if stage == 3:
                tmp = sb.tile([P, C2], dt.float32, tag="tmp1")
                nc.sync.dma_start(tmp[:], feat2_f[:P, :C2])
                nc.sync.dma_start(out2_dram[:P, :], tmp[:])

            # ---- layer-2 edge phase + loss ----
            wce_all = cst.tile([P, BINS_PER_CORE], dt.float32)

            def l2_out(nb, ps_out, rec):
                rec4 = sb.tile([P, H2], dt.float32, tag="rec4")
                nc.vector.tensor_scalar(out=rec4[:], in0=rec[:], scalar1=0.25,
                                        scalar2=None, op0=mybir.AluOpType.mult)
                mm = sb.tile([P, H2, C2], dt.float32, tag="mm")
                nc.vector.tensor_tensor(
                    out=mm[:], in0=ps_out[:].rearrange("p (h c) -> p h c", h=H2),
                    in1=rec4[:, :, None].to_broadcast([P, H2, C2]),
                    op=mybir.AluOpType.mult)
                o2 = sb.tile([P, C2], dt.float32, tag="o2")
                nc.vector.reduce_sum(out=o2[:],
                                     in_=mm[:].rearrange("p h c -> p c h"),
                                     axis=mybir.AxisListType.X)
                nc.vector.tensor_tensor(out=o2[:], in0=o2[:], in1=b2t[:],
                                        op=mybir.AluOpType.add)
                nc.sync.dma_start(out2_dram[nb * P:(nb + 1) * P, :], o2[:])
                # CE loss partials
                mx = sb.tile([P, 1], dt.float32, tag="mx")
                nc.vector.reduce_max(out=mx[:], in_=o2[:],
                                     axis=mybir.AxisListType.X)
                nm = sb.tile([P, 1], dt.float32, tag="nm")
                nc.vector.tensor_scalar(out=nm[:], in0=mx[:], scalar1=-1.0,
                                        scalar2=None, op0=mybir.AluOpType.mult)
                exps = sb.tile([P, C2], dt.float32, tag="exps")
                se = sb.tile([P, 1], dt.float32, tag="se")
                nc.scalar.activation(exps[:], o2[:],
                                     mybir.ActivationFunctionType.Exp,
                                     bias=nm[:, :1], accum_out=se[:])
                lse = sb.tile([P, 1], dt.float32, tag="lse")
                nc.scalar.activation(lse[:], se[:],
                                     mybir.ActivationFunctionType.Ln)
                oh = sb.tile([P, C2], dt.float32, tag="oh")
                nc.vector.tensor_scalar(out=oh[:], in0=iota[:],
                                        scalar1=yloc[:, nb:nb + 1],
                                        scalar2=None,
                                        op0=mybir.AluOpType.is_equal)
                nc.vector.tensor_tensor(out=oh[:], in0=oh[:], in1=o2[:],
                                        op=mybir.AluOpType.mult)
                oy = sb.tile([P, 1], dt.float32, tag="oy")
                nc.vector.reduce_sum(out=oy[:], in_=oh[:],
                                     axis=mybir.AxisListType.X)
                ce = sb.tile([P, 1], dt.float32, tag="ce")
                nc.vector.tensor_tensor(out=ce[:], in0=mx[:], in1=lse[:],
                                        op=mybir.AluOpType.add)
                nc.vector.tensor_tensor(out=ce[:], in0=ce[:], in1=oy[:],
                                        op=mybir.AluOpType.subtract)
                nc.vector.tensor_tensor(out=wce_all[:, nb:nb + 1], in0=ce[:],
                                        in1=wmask[:, nb:nb + 1],
                                        op=mybir.AluOpType.mult)

            if stage >= 4:
                _edge_layer(nc, sb, psb, psd, 2, T, feat2_f, sc2_f, consts,
                            l2_out)
                nc.sync.dma_start(wce_dram[:], wce_all[:])

    nc.compile()
    return nc


_BUILD_CACHE = {}


def kernel(**inputs):
    per_core, shared, meta = _prep(**inputs)
    T = meta["T"]
    if T not in _BUILD_CACHE:
        _BUILD_CACHE[T] = _build(T)
    nc = _BUILD_CACHE[T]

    in_maps = [{**shared, **pc} for pc in per_core]
    trace = bool(int(os.environ.get("GAT_TRACE", "0")))
    res = run_bass_kernel_spmd(nc, in_maps, core_ids=list(range(NCORES)),
                               trace=trace)
    LAST_EXEC_NS[0] = res.exec_time_ns

    node_of_slot = meta["node_of_slot"]
    out = np.zeros((N, C2), np.float32)
    wce_sum = 0.0
    for c in range(NCORES):
        r = res.results[c]
        nos = node_of_slot[c * SLOTS_PER_CORE:(c + 1) * SLOTS_PER_CORE]
        ok = nos >= 0
        out[nos[ok]] = r["out2"][ok]
        wce_sum += float(r["wce"].sum(dtype=np.float64))
    loss = np.float32(wce_sum / float(meta["mask_sum"]))
    return loss, out
